# revision 29
# baseline (speedup 1.0000x reference)
"""Trainium2 Bass kernel for CrossScaleAttention.

Computes, for input x [B=8, C=256, H=48, W=48] (N = H*W = 2304):
    q = Wq x + bq ; k = Wk x + bk ; v = Wv x + bv       (1x1 conv projections)
    per head h (4 heads, d=64): attn = softmax(q_h^T k_h / 8)
    o_h = v_h attn^T ; out = Wo o + bo ; y = x + gamma * out

Sharding: data-parallel over batch; core b handles batch element b.
No collectives; each core loads its slice + replicated weights and
writes its output slice.

Device algorithm per core (matmuls in fp32r: 1 col/cycle at N>=256):
  - Q, K in native [o, n] layout:  Q = WqT^T @ X   (lhsT = Wq^T chunks)
  - V^T directly via  V1T = X^T @ WvT  (lhsT = X chunks) with a ones
    column appended per head (cols h*65+64) so that the attention A@V
    matmul also produces the softmax row-sums for free.
  - Scores computed TRANSPOSED (S^T[m, n] = k^T q) so no PE transposes
    are needed anywhere: lhsT = k chunk [64, 128], rhs = q [64, nb].
  - exp(S^T/8) fused into the PSUM->SBUF evacuation on the scalar
    engine (softmax without max-subtraction: scores are ~N(0,1), safely
    inside fp32 exp range for this input distribution).
  - AV: psum[65, nb] += V1T_chunk[128, 65]^T @ E^T_chunk[128, nb] over
    18 m-chunks; row 64 = softmax denominators.
  - normalize: recip of row 64 (DVE), broadcast to 64 partitions via a
    K=1 PE matmul, multiply on DVE while evacuating PSUM.
  - O projection accumulated per head with K=64 chunks of Wo^T, bias
    added via a K=1 ones matmul, residual fused on DVE:
    y = (proj + bo) * gamma + x.

All tiles that feed PE matmuls are declared float32r (the BIR verifier
requires fp32r operands to be produced as fp32r); DMA loads bitcast the
f32 DRAM side, and compute producers write with fp32r output dtype.
"""

import numpy as np

import concourse.bass as bass
import concourse.mybir as mybir
import concourse.tile as tile

F32 = mybir.dt.float32
F32R = mybir.dt.float32r
AF = mybir.ActivationFunctionType

C = 256
N = 2304  # 48*48
NH = 4
HD = 64  # head dim
NCORES = 8
KC = 128  # contraction chunk
NMC = N // KC  # 18 m-chunks
# n-blocks (free-dim blocks, <=512 for fp32 moving operand)
BLOCKS = [(0, 512), (512, 512), (1024, 512), (1536, 512), (2048, 256)]

_MAX_WAITS = 1  # walrus in this environment accepts 1 sync-wait per instruction


def _split_multi_waits(nc):
    """Hoist excess sem-waits onto same-engine NoOps emitted just before the
    owning instruction (the engine stalls at the NoOp instead — identical
    semantics, one wait per instruction)."""
    n = 0
    for bb in nc.m.functions[0].blocks:
        insts = bb.instructions
        i = 0
        while i < len(insts):
            inst = insts[i]
            si = inst.sync_info
            waits = list(si.on_wait) if si and si.on_wait else []
            if len(waits) > _MAX_WAITS:
                keep = waits[-_MAX_WAITS:]
                extra = waits[: -_MAX_WAITS]
                si.on_wait.clear()
                for w in keep:
                    si.on_wait.append(w)
                nops = []
                while extra:
                    chunk, extra = extra[:_MAX_WAITS], extra[_MAX_WAITS:]
                    nop = mybir.InstNoOp(name=f"I-waitnop-{n}", ins=[], outs=[])
                    n += 1
                    nop.engine = inst.engine
                    nop.sync_info = mybir.SyncInfo(on_wait=chunk, on_update=[])
                    nops.append(nop)
                insts[i:i] = nops
                i += len(nops)
            i += 1


def _fix_unsupported_isa(nc):
    """This walrus build rejects EVENT_SEMAPHORE_RANGE_CLEAR ('ISA wrong
    length'); replace it with per-semaphore write-0 EventSemaphore ops."""
    for bb in nc.m.functions[0].blocks:
        insts = bb.instructions
        idx = 0
        while idx < len(insts):
            i = insts[idx]
            if (
                type(i).__name__ == "InstISA"
                and i.op_name == "EVENT_SEMAPHORE_RANGE_CLEAR"
            ):
                d = i.ant_dict
                waits = list(i.sync_info.on_wait) if i.sync_info and i.sync_info.on_wait else []
                repl = []
                for s in range(d["range_first"], d["range_last"] + 1):
                    ev = mybir.InstEventSemaphore(
                        name=f"I-semclr-{bb.name}-{s}", ins=[], outs=[]
                    )
                    ev.engine = i.engine
                    ev.sync_info = mybir.SyncInfo(
                        on_wait=waits if s == d["range_first"] else [],
                        on_update=[
                            mybir.SyncUpdate(
                                sync_type="semaphore",
                                id=s,
                                ant_name=f"clr{s}",
                                update_mode="sem-wr-imm",
                                update_value=0,
                                update_reg=None,
                            )
                        ],
                    )
                    repl.append(ev)
                insts[idx : idx + 1] = repl
                idx += len(repl)
            else:
                idx += 1


def build_module(for_hw=True):
    nc = bass.Bass()

    x_d = nc.dram_tensor("x", [C, N], F32, kind="ExternalInput")
    wqt_d = nc.dram_tensor("wqt", [C, C], F32, kind="ExternalInput")
    wkt_d = nc.dram_tensor("wkt", [C, C], F32, kind="ExternalInput")
    wvt_d = nc.dram_tensor("wvt", [C, NH * (HD + 1)], F32, kind="ExternalInput")
    wot_d = nc.dram_tensor("wot", [C, C], F32, kind="ExternalInput")
    bq_d = nc.dram_tensor("bq", [C, 1], F32, kind="ExternalInput")
    bk_d = nc.dram_tensor("bk", [C, 1], F32, kind="ExternalInput")
    bvrow_d = nc.dram_tensor("bvrow", [1, NH * (HD + 1)], F32, kind="ExternalInput")
    ones_d = nc.dram_tensor("ones", [128, 512], F32, kind="ExternalInput")
    borow_d = nc.dram_tensor("borow", [1, C], F32, kind="ExternalInput")
    gamma_d = nc.dram_tensor("gamma128", [128, 1], F32, kind="ExternalInput")
    y_d = nc.dram_tensor("y", [C, N], F32, kind="ExternalOutput")

    with tile.TileContext(nc) as tc:
        consts = tc.alloc_tile_pool(name="consts", bufs=1)

        def ctile(shape, dtype, nm):
            return consts.tile(shape, dtype, tag=nm, name=nm)

        # ---- persistent SBUF tensors (f32r = PE matmul operands) ----
        X = [ctile([128, N], F32R, f"x{t}") for t in range(2)]
        Q = [ctile([128, N], F32R, f"q{t}") for t in range(2)]
        K = [ctile([128, N], F32R, f"k{t}") for t in range(2)]
        V1T = [ctile([128, NH * (HD + 1)], F32R, f"v1t{i}") for i in range(NMC)]
        OH = [ctile([128, N], F32R, f"oh{h}") for h in range(NH)]
        WQT = [ctile([128, C], F32R, f"wqt{t}") for t in range(2)]
        WKT = [ctile([128, C], F32R, f"wkt{t}") for t in range(2)]
        WVT = [ctile([128, NH * (HD + 1)], F32R, f"wvt{t}") for t in range(2)]
        WOTH = [ctile([128, C], F32R, f"woth{h}") for h in range(NH)]
        BQ = [ctile([128, 1], F32, f"bq{t}") for t in range(2)]
        BK = [ctile([128, 1], F32, f"bk{t}") for t in range(2)]
        BVROW = ctile([128, NH * (HD + 1)], F32R, "bvrow")
        BOROW = ctile([128, C], F32R, "borow")
        GAMMA = ctile([128, 1], F32, "gamma")
        ONES = ctile([128, 512], F32R, "ones")

        # ---- loads ----
        for t in range(2):
            sl = slice(t * 128, (t + 1) * 128)
            nc.sync.dma_start(out=X[t], in_=x_d[sl, :].bitcast(F32R))
            nc.sync.dma_start(out=WQT[t], in_=wqt_d[sl, :].bitcast(F32R))
            nc.sync.dma_start(out=WKT[t], in_=wkt_d[sl, :].bitcast(F32R))
            nc.sync.dma_start(out=WVT[t], in_=wvt_d[sl, :].bitcast(F32R))
            nc.sync.dma_start(out=BQ[t], in_=bq_d[sl, :])
            nc.sync.dma_start(out=BK[t], in_=bk_d[sl, :])
        for h in range(NH):
            nc.sync.dma_start(
                out=WOTH[h][0:HD, :], in_=wot_d[h * HD : (h + 1) * HD, :].bitcast(F32R)
            )
        nc.sync.dma_start(out=BVROW[0:1, :], in_=bvrow_d[:, :].bitcast(F32R))
        nc.sync.dma_start(out=BOROW[0:1, :], in_=borow_d[:, :].bitcast(F32R))
        nc.sync.dma_start(out=GAMMA, in_=gamma_d[:, :])
        nc.sync.dma_start(out=ONES, in_=ones_d[:, :].bitcast(F32R))

        psum_proj = tc.alloc_tile_pool(name="psp", bufs=2, space="PSUM")

        # ---- stage 1: Q, K projections (native [o, n] layout) ----
        for W2, B2, DST in ((WQT, BQ, Q), (WKT, BK, K)):
            for ot in range(2):
                for n0, nw in BLOCKS:
                    ps = psum_proj.tile([128, 512], F32, tag="psp", name="psp")
                    nc.tensor.matmul(
                        ps[:, :nw],
                        W2[0][:, ot * 128 : (ot + 1) * 128],
                        X[0][:, n0 : n0 + nw],
                        start=True,
                        stop=False,
                    )
                    nc.tensor.matmul(
                        ps[:, :nw],
                        W2[1][:, ot * 128 : (ot + 1) * 128],
                        X[1][:, n0 : n0 + nw],
                        start=False,
                        stop=True,
                    )
                    nc.vector.tensor_scalar_add(
                        DST[ot][:, n0 : n0 + nw], ps[:, :nw], B2[ot]
                    )

        # ---- stage 2: V^T with per-head ones columns ----
        # WVT/BVROW are host-augmented to the V1T layout (4 x [64 data | 1
        # ones]): weight ones-cols are 0 and the bias row carries 1.0 there,
        # so the K=1 bias matmul also writes the ones columns.
        for i in range(NMC):
            ps = psum_proj.tile([128, NH * (HD + 1)], F32, tag="psv", name="psv")
            nc.tensor.matmul(
                ps,
                X[0][:, i * 128 : (i + 1) * 128],
                WVT[0],
                start=True,
                stop=False,
            )
            nc.tensor.matmul(
                ps,
                X[1][:, i * 128 : (i + 1) * 128],
                WVT[1],
                start=False,
                stop=False,
            )
            # bias add: ones[1,128]^T @ bvrow[1,256]
            nc.tensor.matmul(
                ps,
                ONES[0:1, 0:128],
                BVROW[0:1, :],
                start=False,
                stop=True,
            )
            nc.scalar.activation(V1T[i], ps, AF.Copy)

        # ---- stages 3+4: attention + output projection, per n-block ----
        psum_proj.release()
        et_pool = tc.alloc_tile_pool(name="et", bufs=NMC + 2)
        rc_pool = tc.alloc_tile_pool(name="rc", bufs=2)
        bc_pool = tc.alloc_tile_pool(name="bc", bufs=2)
        out_pool = tc.alloc_tile_pool(name="out", bufs=3)
        ps_s = tc.alloc_tile_pool(name="pss", bufs=2, space="PSUM")
        ps_av = tc.alloc_tile_pool(name="psav", bufs=2, space="PSUM")
        ps_bc = tc.alloc_tile_pool(name="psbc", bufs=2, space="PSUM")
        ps_o = tc.alloc_tile_pool(name="pso", bufs=2, space="PSUM")

        for n0, nw in BLOCKS:
            for h in range(NH):
                ht, hp = divmod(h, 2)
                qh = Q[ht][hp * HD : (hp + 1) * HD, n0 : n0 + nw]
                ET = []
                for mc in range(NMC):
                    kh = K[ht][hp * HD : (hp + 1) * HD, mc * 128 : (mc + 1) * 128]
                    ps = ps_s.tile([128, 512], F32, tag="s", name="s")
                    nc.tensor.matmul(ps[:, :nw], kh, qh, start=True, stop=True)
                    et = et_pool.tile([128, 512], F32R, tag="et", name="et")
                    nc.scalar.activation(et[:, :nw], ps[:, :nw], AF.Exp, scale=0.125)
                    ET.append(et)
                psa = ps_av.tile([HD + 1, 512], F32, tag="av", name="av")
                for mc in range(NMC):
                    nc.tensor.matmul(
                        psa[:, :nw],
                        V1T[mc][:, h * (HD + 1) : (h + 1) * (HD + 1)],
                        ET[mc][:, :nw],
                        start=(mc == 0),
                        stop=(mc == NMC - 1),
                    )
                # softmax denominators: copy row 64 to SBUF, broadcast to 64
                # partitions via K=1 matmul, then reciprocal on all 64 lanes
                rc = rc_pool.tile([128, 512], F32R, tag="rc", name="rc")
                nc.vector.tensor_copy(rc[HD : HD + 1, :nw], psa[HD : HD + 1, :nw])
                psb = ps_bc.tile([HD, 512], F32, tag="bc", name="bc")
                nc.tensor.matmul(
                    psb[:, :nw],
                    ONES[HD : HD + 1, 0:HD],
                    rc[HD : HD + 1, :nw],
                    start=True,
                    stop=True,
                )
                bc = bc_pool.tile([HD, 512], F32, tag="bcs", name="bcs")
                nc.vector.reciprocal(bc[:, :nw], psb[:, :nw])
                nc.vector.tensor_mul(
                    OH[h][0:HD, n0 : n0 + nw], psa[0:HD, :nw], bc[:, :nw]
                )
            # output projection for this n-block + fused bias/residual
            for ot in range(2):
                pso = ps_o.tile([128, 512], F32, tag="o", name="o")
                for h in range(NH):
                    nc.tensor.matmul(
                        pso[:, :nw],
                        WOTH[h][0:HD, ot * 128 : (ot + 1) * 128],
                        OH[h][0:HD, n0 : n0 + nw],
                        start=(h == 0),
                        stop=False,
                    )
                # + bo (broadcast over n): lhsT = bo row [1, 128], rhs = ones [1, nw]
                nc.tensor.matmul(
                    pso[:, :nw],
                    BOROW[0:1, ot * 128 : (ot + 1) * 128],
                    ONES[0:1, 0:nw],
                    start=False,
                    stop=True,
                )
                outt = out_pool.tile([128, 512], F32, tag="out", name="out")
                nc.vector.scalar_tensor_tensor(
                    outt[:, :nw],
                    pso[:, :nw],
                    GAMMA,
                    X[ot][:, n0 : n0 + nw].bitcast(F32),
                    op0=mybir.AluOpType.mult,
                    op1=mybir.AluOpType.add,
                )
                nc.sync.dma_start(
                    out=y_d[ot * 128 : (ot + 1) * 128, n0 : n0 + nw],
                    in_=outt[:, :nw],
                )

        for p in (ps_o, ps_bc, ps_av, ps_s, out_pool, bc_pool, rc_pool, et_pool, consts):
            p.release()

    if for_hw:
        # walrus-compat rewrites; CoreSim can't execute post-hoc instructions
        _fix_unsupported_isa(nc)
        _split_multi_waits(nc)
    return nc


def make_in_maps(x, Wq, bq, Wk, bk, Wv, bv, Wo, bo, gamma):
    x = np.asarray(x, dtype=np.float32)
    B = x.shape[0]
    gamma = np.asarray(gamma, dtype=np.float32).reshape(-1)[0]
    bo_arr = np.asarray(bo, np.float32).reshape(C)
    # augment Wv^T / bv to the V1T layout: per head, 64 data cols + 1 ones col
    wvt = np.asarray(Wv, np.float32).T  # [c, o]
    wvt_aug = np.zeros((C, NH * (HD + 1)), np.float32)
    bv_aug = np.zeros((1, NH * (HD + 1)), np.float32)
    bv_arr = np.asarray(bv, np.float32).reshape(C)
    for h in range(NH):
        wvt_aug[:, h * (HD + 1) : h * (HD + 1) + HD] = wvt[:, h * HD : (h + 1) * HD]
        bv_aug[0, h * (HD + 1) : h * (HD + 1) + HD] = bv_arr[h * HD : (h + 1) * HD]
        bv_aug[0, h * (HD + 1) + HD] = 1.0
    common = {
        "wqt": np.ascontiguousarray(np.asarray(Wq, np.float32).T),
        "wkt": np.ascontiguousarray(np.asarray(Wk, np.float32).T),
        "wvt": wvt_aug,
        "wot": np.ascontiguousarray(np.asarray(Wo, np.float32).T),
        "bq": np.asarray(bq, np.float32).reshape(C, 1),
        "bk": np.asarray(bk, np.float32).reshape(C, 1),
        "bvrow": bv_aug,
        "borow": bo_arr.reshape(1, C),
        "ones": np.ones((128, 512), np.float32),
        "gamma128": np.full((128, 1), gamma, np.float32),
    }
    return [
        {"x": np.ascontiguousarray(x[b].reshape(C, -1)), **common} for b in range(B)
    ]


_NC = None


def kernel(x, Wq, bq, Wk, bk, Wv, bv, Wo, bo, gamma):
    global _NC
    from concourse.bass_utils import run_bass_kernel_spmd

    x = np.asarray(x)
    B, Cc, H, W = x.shape
    in_maps = make_in_maps(x, Wq, bq, Wk, bk, Wv, bv, Wo, bo, gamma)
    if _NC is None:
        _NC = build_module()
    res = run_bass_kernel_spmd(_NC, in_maps, core_ids=list(range(len(in_maps))))
    y = np.stack([res.results[b]["y"].reshape(Cc, H, W) for b in range(B)])
    return y.astype(x.dtype)
